# revision 12
# baseline (speedup 1.0000x reference)
"""Trainium2 Bass kernel for nn_DecoderBlock (B=8, N=1024, D=512, H=8, DH=64, DE=2048).

Strategy: 8-way data parallel over batch B — each NeuronCore computes the full
decoder block for one batch element; no collectives.

Algebraic refactors (exact in real arithmetic):
  1. Softmax-free attention is linear:
         out @ W_merge = sum_h q_h (k_h^T h) M_h,   M_h := W_v_h @ W_merge_h
     and the q-projection folds further:
         sum_h q_h sW_h = h @ (alpha*W_q @ SW),  SW := vstack_h(s_h @ M_h)
     so attention+merge is: k = h Wk; sT = h^T k; sW_h = s_h M_h;
     C = (alpha*Wq) @ SW; merged = h @ C.  (b_qkv == 0 for this problem.)
  2. LN1 gain and LN2 mean-centering fold into W_ff1 (host-side).
  3. LN2's variance is computed as a quadratic form *before* ff1:
         ss_j = g1_j^T (W1c W1c^T) g1_j = ||f_j||^2
     with the Gram matrix W1G := W1c @ W1c^T precomputed on host. ss comes out
     seq-major, so rstd2 pre-scales g1 (linearity, b_ff1 == 0) and ff1's PSUM
     is evacuated by a single fused Act op: fT = silu(g2 * psum + b2).

Scheduling for the PE HAM clock gate (cold = half rate; idle >3.4us re-cools):
  - a short warm chain of dummy matmuls runs at t=0 while input DMAs stream;
  - every weight is DMA'd at t=0 in its exact SBUF layout (bf16, single
    contiguous descriptors);
  - the PE instruction stream is ordered so no data dependency leaves a gap:
    LN0 feeds per-chunk transposes+k, then sT/sW/C/merge run dense; ff2
    chases ff1 o-by-o with a lag of 2 (no rstd barrier thanks to refactor 3).
All LN rsqrts run on DVE (Newton iteration, bit-trick seed): the ACT engine
only ever evaluates Silu/Copy, which share one table set -> exactly one
ACT_TABLE_LOAD in the whole kernel (the baseline paid 14).

Everything on-device is bf16 except LN stats, PSUM, x1 and y (f32). Host
converts inputs/weights to bf16 and pre-arranges them in SBUF-native layout.
"""

import numpy as np

_B, _N, _D = 8, 1024, 512
_H, _DH, _DE = 8, 64, 2048
_EPS = 1e-5
_P = 128
_NT = _N // _P      # 8 seq chunks
_KD = _D // _P      # 4 d chunks
_KE = _DE // _P     # 16 d_expand chunks
_NCORES = 8
_RSQRT_MAGIC_F32 = np.int32(0x5F3759DF).view(np.float32) if False else \
    np.frombuffer(np.uint32(0x5F3759DF).tobytes(), dtype=np.float32)[0]


def _patch_tile_drain():
    """Walrus in this container caps sync-waits per TPB_CTRL instruction; the
    stock TileContext exit drain attaches one wait per live proc. Split the
    excess onto single-wait SP nops emitted before the semaphore reset."""
    import bass_rust
    import concourse.tile as tile

    if getattr(tile.TileContext, "_drain_patched", False):
        return

    def _drain_and_barrier(self, tick_clock, wait_clock):
        nc = self.nc
        drain_inst = nc.sync.drain()
        wait_clock.add_sem_waits(
            drain_inst.ins, tile.ScopedClock({None: tick_clock.global_clock})
        )
        si = drain_inst.ins.sync_info
        if si is not None and si.on_wait and len(si.on_wait) > 1:
            waits = list(si.on_wait)
            drain_inst.ins.sync_info = bass_rust.SyncInfo(
                on_wait=[waits[0]], on_update=list(si.on_update or [])
            )
            for w in waits[1:]:
                n = nc.sync.nop()
                n.ins.sync_info = bass_rust.SyncInfo(on_wait=[w], on_update=[])
        nc.all_engine_barrier()
        assert self.sems is not None
        popped = nc._tile_sem_poison_stack.pop()
        assert popped is self._sem_poison
        nc.clear_and_free_semaphores(list(self.sems.allocated().values()))
        nc.all_engine_barrier()

    tile.TileContext._drain_and_barrier = _drain_and_barrier
    tile.TileContext._drain_patched = True


def _split_excess_waits(nc):
    """Walrus codegen caps sync-waits per instruction (2 for EventSemaphore,
    1 otherwise). Tile's sem assigner can exceed that; move excess waits onto
    single-wait nops inserted just before the instruction on the same engine."""
    import bass_rust
    import concourse.mybir as mybir

    for blk in nc.main_func.blocks:
        il = blk.instructions
        i = 0
        while i < len(il):
            ins = il[i]
            si = ins.sync_info
            if si is not None and si.on_wait:
                cap = 2 if type(ins).__name__ == "InstEventSemaphore" else 1
                if len(si.on_wait) > cap:
                    waits = list(si.on_wait)
                    keep, excess = waits[-cap:], waits[:-cap]
                    ins.sync_info = bass_rust.SyncInfo(
                        on_wait=keep, on_update=list(si.on_update or []))
                    for w in excess:
                        nop = mybir.InstNoOp(
                            name=nc.get_next_instruction_name(), ins=[], outs=[])
                        nop.engine = ins.engine
                        nop.sync_info = bass_rust.SyncInfo(
                            on_wait=[w], on_update=[])
                        nc.register_instruction(nop, overwrite=True)
                        il.insert(i, nop)
                        i += 1
            i += 1


def _build_program(flags):
    import concourse.bass as bass
    import concourse.tile as tile
    from concourse import mybir
    from concourse.masks import make_identity

    _patch_tile_drain()

    F32 = mybir.dt.float32
    I32 = mybir.dt.int32
    BF16 = mybir.dt.bfloat16
    Act = mybir.ActivationFunctionType
    Alu = mybir.AluOpType
    P, NT, KD, KE = _P, _NT, _KD, _KE
    NH = _N // 2  # seq half

    nc = bass.Bass()
    needed = []

    def din(name, shape, dt=BF16):
        needed.append(name)
        return nc.declare_dram_parameter(name, list(shape), dt, isOutput=False)

    # All inputs are host-prearranged in exact SBUF layout (partition-first).
    xs = din("xs", (P, NT, _D))            # x[t*128+p, d] -> [p, t, d]
    pos2 = din("pos2", (P, NT, _D))        # pos_enc + ln0_b, same layout
    g0b = None if flags["g0"] else din("g0b", (P, _D))
    wk = din("wk", (P, KD, _D))            # Wk col128: [p, ki, f], d=ki*128+p
    wqT = din("wqT", (P, KD, _D))          # (alpha*Wq)^T col128 over e
    ms = din("ms", (P, _H, KD, _D))        # M_h col128
    w1g = din("w1g", (P, KD, _D))          # W1G = W1c @ W1c^T col128
    wff1 = din("wff1", (P, KD, _DE))       # centered diag(ln1_g) @ W_ff1
    g2c = din("g2c", (P, KE), F32)         # ln2_g col layout
    b2c = din("b2c", (P, KE), F32)         # ln2_b col layout
    wff2 = din("wff2", (P, KE, _D))
    bkb = None if flags["bk"] else din("bkb", (P, _D), F32)
    if not flags["bm"]:
        raise NotImplementedError("nonzero b_merge not supported")
    bf2b = None if flags["bf2"] else din("bf2b", (P, _D), F32)
    yout = nc.declare_dram_parameter("y", [P, NT, _D], F32, isOutput=True)

    def mm(out, lhsT, rhs, start, stop):
        nc.tensor.matmul(out, lhsT, rhs, start=start, stop=stop)

    with tile.TileContext(nc, pool_alloc_mode="queue") as tc:
        with (
            tc.tile_pool(name="persist", bufs=1) as persist,
            tc.tile_pool(name="pt", bufs=2, space="PSUM") as ptp,
            tc.tile_pool(name="pmm", bufs=2, space="PSUM") as pmm,
            tc.tile_pool(name="pacc", bufs=2, space="PSUM") as pacc,
        ):
            # ---- constants (warm tile first: the PE warm chain hangs
            # off this memset, so it must clear the DVE queue early) ----
            warm = persist.tile([P, 512], BF16)
            nc.vector.memset(warm, 0.125)
            ident_f = persist.tile([P, P], F32)
            make_identity(nc, ident_f)
            ident = persist.tile([P, P], BF16)
            nc.vector.tensor_copy(ident[:], ident_f[:])
            magic = persist.tile([P, NT], F32)
            nc.vector.memset(magic, float(_RSQRT_MAGIC_F32))

            def rsqrt(out_sl, v_sl, scratch_tag):
                """out = 1/sqrt(v) elementwise on a [P, w] slice, DVE-only:
                bit-trick seed + 2 Newton iterations (~5e-6 rel err)."""
                w = v_sl.shape[1]
                t0 = persist.tile([P, NT], F32, tag=f"rs{scratch_tag}0",
                                  name=f"rs{scratch_tag}0")
                t1 = persist.tile([P, NT], F32, tag=f"rs{scratch_tag}1",
                                  name=f"rs{scratch_tag}1")
                a, b = t0[:, :w], t1[:, :w]
                # seed: y0 = bitcast(magic - (bitcast(v) >> 1))
                nc.vector.tensor_scalar(
                    a.bitcast(I32), v_sl.bitcast(I32), 1, None,
                    op0=Alu.logical_shift_right)
                nc.vector.tensor_tensor(
                    out_sl.bitcast(I32), magic[:, :w].bitcast(I32),
                    a.bitcast(I32), op=Alu.subtract)
                for _ in range(2):  # y *= 1.5 - 0.5*v*y*y
                    nc.vector.tensor_tensor(a, out_sl, out_sl, op=Alu.mult)
                    nc.vector.tensor_tensor(b, a, v_sl, op=Alu.mult)
                    nc.vector.tensor_scalar(a, b, -0.5, 1.5,
                                            op0=Alu.mult, op1=Alu.add)
                    nc.vector.tensor_tensor(out_sl, out_sl, a, op=Alu.mult)

            # ---- PE warm chain: release the HAM clock gate while DMAs run --
            pwarm = pmm.tile([P, 512], F32, tag="mm", name="pwarm")
            for w in range(14):
                mm(pwarm[:], warm[:, :128], warm[:],
                   start=(w == 0), stop=(w == 13))

            x1_t = persist.tile([P, NT, _D], F32)
            x_t = persist.tile([P, NT, _D], BF16)
            pos_t = persist.tile([P, NT, _D], BF16)
            # Input DMAs split in halves so LN0 starts ~3us in; each
            # dma_start costs ~0.7us of SP issue time, so few big transfers
            # beat many small ones.
            nc.sync.dma_start(x_t[:, 0:2, :], xs[:, 0:2, :])
            nc.sync.dma_start(pos_t[:, 0:2, :], pos2[:, 0:2, :])
            nc.sync.dma_start(x_t[:, 2:4, :], xs[:, 2:4, :])
            nc.sync.dma_start(pos_t[:, 2:4, :], pos2[:, 2:4, :])

            # ---------------- Phase A: LN0 + attention + merge ----------------
            with (
                tc.tile_pool(name="phA", bufs=1) as A,
                tc.tile_pool(name="lnp", bufs=4) as lnp,
            ):
                wk_t = A.tile([P, KD, _D], BF16)
                nc.sync.dma_start(wk_t[:], wk[:, :, :])
                wqT_t = A.tile([P, KD, _D], BF16)
                nc.sync.dma_start(wqT_t[:], wqT[:, :, :])
                nc.sync.dma_start(x_t[:, 4:8, :], xs[:, 4:8, :])
                nc.sync.dma_start(pos_t[:, 4:8, :], pos2[:, 4:8, :])
                m_t = A.tile([P, _H, KD, _D], BF16)
                nc.sync.dma_start(m_t[:], ms[:, :, :, :])
                if g0b is not None:
                    g0_t = A.tile([P, _D], F32)
                    nc.sync.dma_start(g0_t[:], g0b[:, :])
                if bkb is not None:
                    bk_t = A.tile([P, _D], F32)
                    nc.sync.dma_start(bk_t[:], bkb[:, :])

                h_t = A.tile([P, NT, _D], BF16)
                hT_t = A.tile([P, KD, _N], BF16)
                k_t = A.tile([P, NT, _D], BF16)
                sT_t = A.tile([P, KD, _D], BF16)
                sw_ts = [A.tile([P, _D], BF16, tag=f"sw{j}", name=f"sw{j}")
                         for j in range(_H // 2)]
                C_t = A.tile([P, KD, _D], BF16)
                mv0 = A.tile([P, NT, 2], F32)
                rstd0 = A.tile([P, NT], F32)

                # LN0 stats (DVE), rsqrt per chunk-pair (DVE Newton):
                # small batches so the first transpose lands ~3us in
                for pair in range(4):
                    for tt in range(2):
                        t = pair * 2 + tt
                        st = lnp.tile([P, 6], F32, tag="st")
                        nc.vector.bn_stats(st[:], x_t[:, t, :])
                        nc.vector.bn_aggr(mv0[:, t, :], st[:])
                    sl = slice(pair * 2, pair * 2 + 2)
                    v = lnp.tile([P, 2], F32, tag="v0", name=f"v0{pair}")
                    nc.vector.tensor_scalar(v[:], mv0[:, sl, 1], _EPS, None,
                                            op0=Alu.add)
                    rsqrt(rstd0[:, sl], v[:], "a")
                    # apply + pos + swish -> h ; transpose -> hT ; k matmuls
                    for tt in range(2):
                        t = pair * 2 + tt
                        tmp = lnp.tile([P, _D], BF16, tag="tmp", name="tmp")
                        nc.vector.tensor_scalar(
                            tmp[:], x_t[:, t, :], mv0[:, t, 0:1],
                            rstd0[:, t:t + 1],
                            op0=Alu.subtract, op1=Alu.mult)
                        if g0b is not None:
                            nc.vector.tensor_mul(tmp[:], tmp[:], g0_t[:])
                        nc.gpsimd.tensor_add(tmp[:], tmp[:], pos_t[:, t, :])
                        nc.scalar.activation(h_t[:, t, :], tmp[:], Act.Silu)
                        pt = ptp.tile([P, 4 * P], BF16, tag="pt", name="ptT")
                        for o in range(KD):
                            nc.tensor.transpose(
                                pt[:, o * P:(o + 1) * P],
                                h_t[:, t, o * P:(o + 1) * P], ident[:])
                        nc.vector.tensor_copy(
                            hT_t[:, :, t * P:(t + 1) * P],
                            pt[:].rearrange("p (o n) -> p o n", n=P))
                        pk = pmm.tile([P, 512], F32, tag="mm", name="pk")
                        for ki in range(KD):
                            mm(pk[:], hT_t[:, ki, t * P:(t + 1) * P],
                               wk_t[:, ki, :],
                               start=(ki == 0), stop=(ki == KD - 1))
                        if bkb is not None:
                            nc.vector.tensor_add(k_t[:, t, :], pk[:], bk_t[:])
                        else:
                            nc.scalar.copy(k_t[:, t, :], pk[:])

                # sT = h^T @ k : [d, head*64]
                for o in range(KD):
                    ps = pmm.tile([P, 512], F32, tag="mm", name="ps")
                    for t in range(NT):
                        mm(ps[:], h_t[:, t, o * P:(o + 1) * P], k_t[:, t, :],
                           start=(t == 0), stop=(t == NT - 1))
                    nc.scalar.copy(sT_t[:, o, :], ps[:])

                # sW_h = s_h @ M_h (64x512); pairs stacked via partition-
                # shifted copyout
                for h_idx in range(_H):
                    pw = pmm.tile([P, 512], F32, tag="mm", name="pw")
                    for ki in range(KD):
                        mm(pw[:64, :],
                           sT_t[:, ki, h_idx * 64:(h_idx + 1) * 64],
                           m_t[:, h_idx, ki, :],
                           start=(ki == 0), stop=(ki == KD - 1))
                    lo = 64 * (h_idx % 2)
                    nc.scalar.copy(sw_ts[h_idx // 2][lo:lo + 64, :],
                                   pw[:64, :])

                # C = (alpha*Wq) @ SW
                for dc in range(KD):
                    pc = pmm.tile([P, 512], F32, tag="mm", name="pc")
                    for ec in range(KD):
                        mm(pc[:], wqT_t[:, ec, dc * P:(dc + 1) * P],
                           sw_ts[ec][:],
                           start=(ec == 0), stop=(ec == KD - 1))
                    nc.scalar.copy(C_t[:, dc, :], pc[:])

                # merged + residual via identity-matmul -> x1 in one PSUM
                # chain; Act evacuates, DVE computes LN1 stats behind it.
                mv1 = persist.tile([P, NT, 2], F32)
                for t in range(NT):
                    pm = pmm.tile([P, 512], F32, tag="mm", name="pm")
                    mm(pm[:], ident[:], x_t[:, t, :], start=True, stop=False)
                    for dc in range(KD):
                        mm(pm[:], hT_t[:, dc, t * P:(t + 1) * P], C_t[:, dc, :],
                           start=False, stop=(dc == KD - 1))
                    x1c = x1_t[:, t, :]
                    nc.scalar.copy(x1c, pm[:])
                    st = lnp.tile([P, 6], F32, tag="st")
                    nc.vector.bn_stats(st[:], x1c)
                    nc.vector.bn_aggr(mv1[:, t, :], st[:])

            # ---------------- Phase B: LN1 + FF ----------------
            with (
                tc.tile_pool(name="phB", bufs=1) as Bp,
                tc.tile_pool(name="lnq", bufs=4) as lnq,
            ):
                w1g_t = Bp.tile([P, KD, _D], BF16)
                nc.sync.dma_start(w1g_t[:], w1g[:, :, :])
                wff1_t = Bp.tile([P, KD, _DE], BF16)
                nc.sync.dma_start(wff1_t[:], wff1[:, :, :])
                wff2_t = Bp.tile([P, KE, _D], BF16)
                nc.sync.dma_start(wff2_t[:], wff2[:, :, :])
                g2_t = Bp.tile([P, KE], F32)
                nc.sync.dma_start(g2_t[:], g2c[:, :])
                b2_t = Bp.tile([P, KE], F32)
                nc.sync.dma_start(b2_t[:], b2c[:, :])
                if bf2b is not None:
                    bf2_t = Bp.tile([P, _D], F32)
                    nc.sync.dma_start(bf2_t[:], bf2b[:, :])

                rstd1 = Bp.tile([P, NT], F32)
                g1_t = Bp.tile([P, NT, _D], BF16)
                g1T_ts = [Bp.tile([P, KD, NH], BF16, tag=f"g1T{s}",
                                  name=f"g1T{s}") for s in range(2)]
                ssv = Bp.tile([P, NT], F32)
                rstd2 = Bp.tile([P, NT], F32)
                g1sT_ts = [Bp.tile([P, KD, NH], BF16, tag=f"g1sT{s}",
                                   name=f"g1sT{s}") for s in range(2)]
                fT_ts = [Bp.tile([P, KE, NH], BF16, tag=f"fT{s}",
                                 name=f"fT{s}") for s in range(2)]

                # LN1 apply + transpose, per half-batch rsqrt
                for half in range(2):
                    sl = slice(half * 4, half * 4 + 4)
                    v = lnq.tile([P, 4], F32, tag="v1", name=f"v1{half}")
                    nc.vector.tensor_scalar(v[:], mv1[:, sl, 1], _EPS, None,
                                            op0=Alu.add)
                    rsqrt(rstd1[:, sl], v[:], "b")
                    for tt in range(4):
                        t = half * 4 + tt
                        nc.vector.tensor_scalar(
                            g1_t[:, t, :], x1_t[:, t, :], mv1[:, t, 0:1],
                            rstd1[:, t:t + 1],
                            op0=Alu.subtract, op1=Alu.mult)
                        pt = ptp.tile([P, 4 * P], BF16, tag="pt", name="ptG")
                        for o in range(KD):
                            nc.tensor.transpose(
                                pt[:, o * P:(o + 1) * P],
                                g1_t[:, t, o * P:(o + 1) * P], ident[:])
                        nc.scalar.copy(
                            g1T_ts[half][:, :, tt * P:(tt + 1) * P],
                            pt[:].rearrange("p (o n) -> p o n", n=P))

                # ss_j = g1_j^T W1G g1_j  (seq-major!), then g1s = g1 * rstd2
                # and transpose. One fused DVE op per chunk: the product
                # writes a scratch tile, accum_out delivers the row sums.
                for half in range(2):
                    for tt in range(4):
                        t = half * 4 + tt
                        pu = pmm.tile([P, 512], F32, tag="mm", name="pu")
                        for ki in range(KD):
                            mm(pu[:], g1T_ts[half][:, ki, tt * P:(tt + 1) * P],
                               w1g_t[:, ki, :],
                               start=(ki == 0), stop=(ki == KD - 1))
                        sscr = lnq.tile([P, _D], BF16, tag="sscr",
                                        name="sscr")
                        with nc.allow_low_precision(
                                reason="LN2 stats: bf16 quadratic form, "
                                       "~0.4% on rstd2 is within tolerance"):
                            nc.vector.scalar_tensor_tensor(
                                sscr[:], pu[:], 1.0, g1_t[:, t, :],
                                op0=Alu.mult, op1=Alu.mult,
                                accum_out=ssv[:, t:t + 1])
                def ln2_scale_half(half):
                    sl = slice(half * 4, half * 4 + 4)
                    v2 = lnq.tile([P, 4], F32, tag="v2", name=f"v2{half}")
                    nc.vector.tensor_scalar(v2[:], ssv[:, sl], 1.0 / _DE,
                                            _EPS, op0=Alu.mult, op1=Alu.add)
                    rsqrt(rstd2[:, sl], v2[:], "c")
                    for tt in range(4):
                        t = half * 4 + tt
                        nc.vector.tensor_scalar(
                            g1_t[:, t, :], g1_t[:, t, :], rstd2[:, t:t + 1],
                            None, op0=Alu.mult)

                def trg1s_half(half):
                    for tt in range(4):
                        t = half * 4 + tt
                        pt = ptp.tile([P, 4 * P], BF16, tag="pt", name="ptS")
                        for o in range(KD):
                            nc.tensor.transpose(
                                pt[:, o * P:(o + 1) * P],
                                g1_t[:, t, o * P:(o + 1) * P], ident[:])
                        nc.scalar.copy(
                            g1sT_ts[half][:, :, tt * P:(tt + 1) * P],
                            pt[:].rearrange("p (o n) -> p o n", n=P))

                # ff1 for half 0; then ff2(h0) tt-major interleaved with
                # ff1(h1); then ff2(h1) tt-major. tt-major accumulation
                # finishes one seq chunk at a time so y streams out early.
                def ff1_step(half, o):
                    pf = pmm.tile([P, 512], F32, tag="mm", name="pf")
                    for ki in range(KD):
                        mm(pf[:], wff1_t[:, ki, o * P:(o + 1) * P],
                           g1sT_ts[half][:, ki, :],
                           start=(ki == 0), stop=(ki == KD - 1))
                    # fT = silu(g2 * f_scaled + b2), one fused Act op
                    nc.scalar.activation(
                        fT_ts[half][:, o, :], pf[:], Act.Silu,
                        bias=b2_t[:, o:o + 1], scale=g2_t[:, o:o + 1])

                accs = {}

                def ff2_quarter(half, tt, oq):
                    if oq == 0:
                        accs[(half, tt)] = pacc.tile(
                            [P, 512], F32, tag="acc", name=f"po{half}_{tt}")
                    acc = accs[(half, tt)]
                    for o in range(oq * 4, oq * 4 + 4):
                        mm(acc[:], fT_ts[half][:, o, tt * P:(tt + 1) * P],
                           wff2_t[:, o, :],
                           start=(o == 0), stop=(o == KE - 1))

                def emit_y(half, tt):
                    t = half * 4 + tt
                    oc = lnq.tile([P, _D], F32, tag="oc", name="oc")
                    nc.vector.tensor_add(oc[:], accs[(half, tt)][:],
                                         x1_t[:, t, :])
                    if bf2b is not None:
                        nc.vector.tensor_add(oc[:], oc[:], bf2_t[:])
                    nc.sync.dma_start(yout[:, t, :], oc[:])

                # h0's scale+transpose gates ff1(h0); h1's LN2 chain (DVE)
                # drains while ff1(h0) owns the PE, and its transposes slot
                # in right after.
                ln2_scale_half(0)
                trg1s_half(0)
                ln2_scale_half(1)
                for o in range(KE):
                    ff1_step(0, o)
                trg1s_half(1)
                for s in range(KE):
                    ff1_step(1, s)
                    tt, oq = s // 4, s % 4
                    ff2_quarter(0, tt, oq)
                    if oq == 3:
                        emit_y(0, tt)
                for tt in range(4):
                    for oq in range(4):
                        ff2_quarter(1, tt, oq)
                    emit_y(1, tt)

    _split_excess_waits(nc)
    return nc, needed


def _host_fold(inputs):
    """Precompute weight layouts/folds. Returns (arrays, flags)."""
    import ml_dtypes
    f32 = np.float32
    bf16 = ml_dtypes.bfloat16
    W_qkv = np.asarray(inputs["W_qkv"], f32)
    b_qkv = np.asarray(inputs["b_qkv"], f32)
    W_merge = np.asarray(inputs["W_merge"], f32)
    alpha = float(np.asarray(inputs["scale"])) ** -0.5

    P = _P

    def col128(w, dt=bf16):  # (D, F) -> (128, D//128, F), d = ki*128 + p
        d, f = w.shape
        return np.ascontiguousarray(
            w.reshape(d // P, P, f).transpose(1, 0, 2).astype(dt))

    def colvec(v):  # (F,) -> (128, F//128), f = o*128 + p
        return np.ascontiguousarray(v.reshape(-1, P).T.astype(f32))

    def bcast(v):  # (D,) -> (128, D)
        return np.ascontiguousarray(
            np.broadcast_to(v, (P, v.shape[0])).astype(f32))

    def seqmajor(a, dt=bf16):  # (N, D) -> (128, NT, D), n = t*128 + p
        return np.ascontiguousarray(
            a.reshape(_NT, P, _D).transpose(1, 0, 2).astype(dt))

    Wq = np.ascontiguousarray(W_qkv[:, :_D]) * f32(alpha)
    Wk = np.ascontiguousarray(W_qkv[:, _D:2 * _D])
    bq = b_qkv[:_D] * f32(alpha)
    bk = b_qkv[_D:2 * _D]
    bv = b_qkv[2 * _D:]
    # v-slice bias must be zero for the M_h fold; q bias must be zero for
    # the C fold (both hold: setup_inputs zeroes b_qkv).
    if np.any(bq != 0.0) or np.any(bv != 0.0):
        raise NotImplementedError("nonzero q/v bias not supported")

    Wv = W_qkv[:, 2 * _D:].reshape(_D, _H, _D)
    M = np.empty((P, _H, _KD, _D), bf16)
    Wm64 = W_merge.astype(np.float64).reshape(_H, _D, _D)
    for h in range(_H):
        mh = (Wv[:, h, :].astype(np.float64) @ Wm64[h]).astype(f32)
        M[:, h] = col128(mh)

    ln0_g = np.asarray(inputs["ln0_g"], f32)
    ln1_g = np.asarray(inputs["ln1_g"], np.float64)
    ln1_b = np.asarray(inputs["ln1_b"], np.float64)
    W_ff1 = np.asarray(inputs["W_ff1"], np.float64)
    w1 = ln1_g[:, None] * W_ff1
    b1 = np.asarray(inputs["b_ff1"], np.float64) + ln1_b @ W_ff1
    # Center so the ff1 matmul emits LN2-pre-centered activations
    w1c = w1 - w1.mean(axis=1, keepdims=True)
    b1c = (b1 - b1.mean()).astype(f32)
    if np.any(np.abs(b1c) > 1e-12):
        raise NotImplementedError("nonzero centered ff1 bias not supported")
    # Gram matrix for the LN2 quadratic form (seq-major sum of squares)
    w1gram = (w1c @ w1c.T).astype(f32)

    b_merge = np.asarray(inputs["b_merge"], f32)
    b_ff2 = np.asarray(inputs["b_ff2"], f32)

    pos2 = (np.asarray(inputs["pos_enc"], f32).reshape(_N, _D)
            + np.asarray(inputs["ln0_b"], f32))

    if np.any(b_merge != 0.0):
        raise NotImplementedError("nonzero b_merge not supported")
    flags = {
        "g0": bool(np.all(ln0_g == 1.0)),
        "bk": bool(np.all(bk == 0.0)),
        "bm": True,
        "bf2": bool(np.all(b_ff2 == 0.0)),
    }

    arrays = {
        "pos2": seqmajor(pos2),
        "g0b": bcast(ln0_g),
        "wk": col128(Wk),
        "wqT": col128(np.ascontiguousarray(Wq.T)),
        "ms": M,
        "w1g": col128(w1gram),
        "wff1": col128(w1c.astype(f32)),
        "g2c": colvec(np.asarray(inputs["ln2_g"], f32)),
        "b2c": colvec(np.asarray(inputs["ln2_b"], f32)),
        "wff2": col128(np.asarray(inputs["W_ff2"], f32)),
        "bkb": bcast(bk),
        "bf2b": bcast(b_ff2),
    }
    return arrays, flags


_PROGRAM_CACHE = {}


def _get_program(flags):
    key = tuple(sorted(flags.items()))
    if key not in _PROGRAM_CACHE:
        _PROGRAM_CACHE[key] = _build_program(flags)
    return _PROGRAM_CACHE[key]


def _stage_inputs(inputs):
    import ml_dtypes
    x = np.asarray(inputs["x"], np.float32)
    arrays, flags = _host_fold(inputs)
    nc, needed = _get_program(flags)
    shared = {k: arrays[k] for k in needed if k != "xs"}
    in_maps = []
    for core in range(_NCORES):
        m = dict(shared)
        xc = x[core].reshape(_NT, _P, _D).transpose(1, 0, 2)
        m["xs"] = np.ascontiguousarray(xc.astype(ml_dtypes.bfloat16))
        in_maps.append(m)
    return nc, in_maps


def kernel(**inputs):
    from concourse.bass_utils import run_bass_kernel_spmd

    nc, in_maps = _stage_inputs(inputs)
    res = run_bass_kernel_spmd(nc, in_maps, core_ids=list(range(_NCORES)))
    # y comes back [128, NT, D] seq-major; invert to [N, D]
    out = np.stack(
        [r["y"].transpose(1, 0, 2).reshape(_N, _D) for r in res.results],
        axis=0)
    return out.astype(np.float32)
